# revision 15
# baseline (speedup 1.0000x reference)
"""MultiHeadDepthwiseSelfAttention Trainium2 kernel (8-core data-parallel over batch).

Math (per batch): q/k/v = depthwise-conv1d(x) (K=3, per-channel, zero pad);
heads of D=64; scores = softmax((q k^T)/sqrt(768)); out = (scores v) @ wo.T + bo.

Per-core design (2 batches), shaped by the TimelineSim cost model:
- All DRAM traffic is contiguous (x loaded token-major, out stored token-major);
  channel-major views are produced by cheap PE transposes instead of 4-byte
  strided DMA access patterns (which cost ~28us each in the DMA model).
- Depthwise conv runs channel-major as 3 per-partition-scalar taps, split
  across DVE / Pool(gpsimd) / Act so no single engine owns it; x^T and v^T
  transposes round-trip through one shared PSUM ring.
- Attention per 2-head pair: scores^T via PE (f32r), exp on Act (the pacing
  engine, ~570ns per [128,512] tile), attn^T accumulated with an augmented
  ones-column in v so the softmax denominator r falls out as PSUM row 64.
  1/r: DVE reciprocal (partition 64 -> 0) then gpsimd partition_broadcast;
  the odd head's normalize writes SBUF partitions 64..127 directly via DVE
  partition shift (no stack DMA).
- Output projection token-major in bf16 (free-256 segment needs bf16's
  1 cyc/row); bias folded in via a broadcast-bias stt eviction (mid-stream)
  or a ones-row matmul (tail blocks, when Act is idle).
- Emission order is hand-pipelined for the in-order engine queues: batch-0
  x/v conv first, then attention(0) interleaved with batch-0 q/k conv and
  batch-1 x/v conv; attention(1) interleaved with batch-1 q/k conv and the
  batch-0 output projection threaded into PE-queue gaps between score
  matmuls; batch-1 projection drains in the tail through the score ring.
"""

import sys

sys.path.insert(0, "/opt/trn_rl_repo")

from contextlib import ExitStack

import numpy as np

import concourse.bass as bass
import concourse.tile as tile
from concourse import bacc, mybir
from concourse.masks import make_identity

F32 = mybir.dt.float32
F32R = mybir.dt.float32r
BF16 = mybir.dt.bfloat16

B, N, FEAT, HEAD, D, KS = 16, 512, 768, 12, 64, 3
NCORES = 8
B_LOC = B // NCORES          # batches per core
NCH = FEAT // 128            # 6 channel chunks (2 heads each)
NJB = N // 128               # 4 token blocks
MUL = mybir.AluOpType.mult
ADD = mybir.AluOpType.add

_PROG_CACHE = {}


def r32(ap):
    return ap.bitcast(F32R)


def _conv3(eng0, eng, out_ap, xt, mid, w_sb, b_sb, c):
    """out = w0*x[n-1] + w1*x[n] + w2*x[n+1] + b  (channel-major chunk c).

    xt is [128, 514] with zero pad at cols 0 and 513. tap0 runs on eng0
    (Pool-capable: plain tensor_scalar); the two accumulating taps on eng."""
    eng0.tensor_scalar(
        mid[:, :], xt[:, 0:N], w_sb[:, c, 0:1], b_sb[:, c, 0:1], MUL, ADD
    )
    eng.scalar_tensor_tensor(
        out=mid[:, :], in0=xt[:, 1 : N + 1], scalar=w_sb[:, c, 1:2],
        in1=mid[:, :], op0=MUL, op1=ADD,
    )
    eng.scalar_tensor_tensor(
        out=out_ap, in0=xt[:, 2 : N + 2], scalar=w_sb[:, c, 2:3],
        in1=mid[:, :], op0=MUL, op1=ADD,
    )


def build_program():
    if "nc" in _PROG_CACHE:
        return _PROG_CACHE["nc"]
    nc = bacc.Bacc("TRN2", target_bir_lowering=False)

    x_d = nc.dram_tensor("x", [B_LOC, N, FEAT], F32, kind="ExternalInput")
    cw_d = nc.dram_tensor("cw", [128, NCH, 12], F32, kind="ExternalInput")
    woT_d = nc.dram_tensor("woT", [FEAT, FEAT], BF16, kind="ExternalInput")
    bo_d = nc.dram_tensor("bo", [1, FEAT], BF16, kind="ExternalInput")
    out_d = nc.dram_tensor("out", [B_LOC, N, FEAT], F32, kind="ExternalOutput")

    with tile.TileContext(nc) as tc, ExitStack() as ctx:
        consts = ctx.enter_context(tc.tile_pool(name="consts", bufs=1))
        xtok_pool = ctx.enter_context(tc.tile_pool(name="xtok", bufs=4))
        xc_pool = ctx.enter_context(tc.tile_pool(name="xchunk", bufs=3))
        xt_pool = ctx.enter_context(tc.tile_pool(name="xt", bufs=8))
        ct_pool = ctx.enter_context(tc.tile_pool(name="convtmp", bufs=2))
        q_pool = ctx.enter_context(tc.tile_pool(name="qT", bufs=12))
        k_pool = ctx.enter_context(tc.tile_pool(name="kT", bufs=12))
        vt_pool = ctx.enter_context(tc.tile_pool(name="vT", bufs=7))
        va_pool = ctx.enter_context(tc.tile_pool(name="vaug", bufs=7))
        exp_pool = ctx.enter_context(tc.tile_pool(name="exp", bufs=9))
        rr_pool = ctx.enter_context(tc.tile_pool(name="rrow", bufs=2))
        bs_pool = ctx.enter_context(tc.tile_pool(name="brc_sb", bufs=2))
        at_pool = ctx.enter_context(tc.tile_pool(name="attnT", bufs=12))
        ot_pool = ctx.enter_context(tc.tile_pool(name="outT", bufs=3))
        # PSUM: sc ring (1-bank tiles) shared by x-transposes, v-transposes,
        # scores and 1/r broadcast; attn banks double-buffered; big = out proj.
        ps_sc = ctx.enter_context(tc.tile_pool(name="ps_sc", bufs=4, space="PSUM"))
        ps_attn = ctx.enter_context(tc.tile_pool(name="ps_attn", bufs=2, space="PSUM"))
        ps_big = ctx.enter_context(tc.tile_pool(name="ps_big", bufs=1, space="PSUM"))

        # constants / weights
        ident_tmp = consts.tile([128, 128], F32)
        make_identity(nc, ident_tmp[:, :])
        ident_f = consts.tile([128, 128], F32)
        nc.vector.tensor_copy(out=r32(ident_f[:, :]), in_=ident_tmp[:, :])
        ones_c = consts.tile([128, HEAD, 1], F32)  # v_aug ones column source
        nc.vector.memset(ones_c[...], 1.0)
        ones_row = consts.tile([1, 128], BF16)      # bias matmul lhsT
        nc.vector.memset(ones_row[...], 1.0)

        x_ap = x_d.ap()
        out_ap = out_d.ap()

        # ---------- emission helpers (in-order engine queues => emission
        # order must match the desired execution timeline) ----------

        def load_x(b):
            xtok = []
            for nb in range(NJB):
                xb = xtok_pool.tile([128, FEAT], F32)
                src = bass.AP(
                    tensor=x_ap.tensor,
                    offset=b * N * FEAT + nb * 128 * FEAT,
                    ap=[[FEAT, 128], [1, FEAT]],
                )
                nc.sync.dma_start(out=r32(xb[:, :]), in_=src.bitcast(F32R))
                xtok.append(xb)
            return xtok

        def load_x_chunk(b, c):
            # one channel chunk across all token blocks: [tok128, nb, ch128];
            # 512B contiguous runs land ~4x sooner than whole-batch loads, so
            # the first chunk's transposes start almost immediately
            xc = xc_pool.tile([128, NJB, 128], F32)
            src = bass.AP(
                tensor=x_ap.tensor,
                offset=b * N * FEAT + c * 128,
                ap=[[FEAT, 128], [128 * FEAT, NJB], [1, 128]],
            )
            nc.sync.dma_start(out=r32(xc[...]), in_=src.bitcast(F32R))
            return xc

        def conv_xv(xtok, c, vT, xts, use_act, evict_act=None):
            """x^T transpose for chunk c + depthwise v-conv; stores the padded
            x^T tile in xts for the later q/k convs."""
            xps = ps_big.tile([128, N], F32, tag="big")
            chunk_major = not isinstance(xtok, list)
            for nb in range(NJB):
                blk = (xtok[:, nb, :] if chunk_major
                       else xtok[nb][:, c * 128 : (c + 1) * 128])
                nc.tensor.transpose(
                    out=r32(xps[:, nb * 128 : (nb + 1) * 128]),
                    in_=r32(blk),
                    identity=r32(ident_f[:, :]),
                )
            xt = xt_pool.tile([128, N + 2], F32)
            nc.gpsimd.memset(xt[:, 0:1], 0.0)
            nc.gpsimd.memset(xt[:, N + 1 : N + 2], 0.0)
            if use_act if evict_act is None else evict_act:
                nc.scalar.copy(out=xt[:, 1 : N + 1], in_=xps[:, :])
            else:
                nc.vector.tensor_copy(out=xt[:, 1 : N + 1], in_=xps[:, :])
            vt = vt_pool.tile([128, N], F32)
            midv = ct_pool.tile([128, N], F32, tag="midv")
            pv = ct_pool.tile([128, N], F32, tag="p2")
            nc.gpsimd.tensor_scalar(
                midv[:, :], xt[:, 0:N], wv_sb[:, c, 0:1], bv_sb[:, c, 0:1],
                MUL, ADD,
            )
            if use_act:
                nc.scalar.activation(
                    out=pv[:, :], in_=xt[:, 2 : N + 2],
                    func=mybir.ActivationFunctionType.Copy,
                    scale=wv_sb[:, c, 2:3],
                )
            else:
                nc.gpsimd.tensor_scalar(
                    pv[:, :], xt[:, 2 : N + 2], wv_sb[:, c, 2:3], None, MUL,
                )
            nc.vector.scalar_tensor_tensor(
                out=midv[:, :], in0=xt[:, 1 : N + 1], scalar=wv_sb[:, c, 1:2],
                in1=midv[:, :], op0=MUL, op1=ADD,
            )
            nc.gpsimd.tensor_add(r32(vt[:, :]), midv[:, :], pv[:, :])
            vT.append(vt)
            xts.append(xt)

        def conv_qk(xts, c, qT, kT, use_act):
            xt = xts[c]
            qt = q_pool.tile([128, N], F32)
            kt = k_pool.tile([128, N], F32)
            midq = ct_pool.tile([128, N], F32, tag="midq")
            midk = ct_pool.tile([128, N], F32, tag="midk")
            if use_act:
                nc.scalar.activation(
                    out=midq[:, :], in_=xt[:, 0:N],
                    func=mybir.ActivationFunctionType.Identity,
                    bias=bq_sb[:, c, 0:1], scale=wq_sb[:, c, 0:1],
                )
                nc.vector.scalar_tensor_tensor(
                    out=midq[:, :], in0=xt[:, 1 : N + 1], scalar=wq_sb[:, c, 1:2],
                    in1=midq[:, :], op0=MUL, op1=ADD,
                )
                nc.vector.scalar_tensor_tensor(
                    out=r32(qt[:, :]), in0=xt[:, 2 : N + 2], scalar=wq_sb[:, c, 2:3],
                    in1=midq[:, :], op0=MUL, op1=ADD,
                )
                nc.gpsimd.tensor_scalar(
                    midk[:, :], xt[:, 0:N], wk_sb[:, c, 0:1],
                    bk_sb[:, c, 0:1], MUL, ADD,
                )
                p2 = ct_pool.tile([128, N], F32, tag="p2")
                nc.scalar.activation(
                    out=p2[:, :], in_=xt[:, 2 : N + 2],
                    func=mybir.ActivationFunctionType.Copy,
                    scale=wk_sb[:, c, 2:3],
                )
                nc.vector.scalar_tensor_tensor(
                    out=midk[:, :], in0=xt[:, 1 : N + 1], scalar=wk_sb[:, c, 1:2],
                    in1=midk[:, :], op0=MUL, op1=ADD,
                )
                nc.vector.tensor_add(r32(kt[:, :]), midk[:, :], p2[:, :])
            else:
                _conv3(nc.gpsimd, nc.vector, r32(qt[:, :]), xt, midq, wq_sb, bq_sb, c)
                nc.gpsimd.tensor_scalar(
                    midk[:, :], xt[:, 0:N], wk_sb[:, c, 0:1],
                    bk_sb[:, c, 0:1], MUL, ADD,
                )
                p2 = ct_pool.tile([128, N], F32, tag="p2")
                nc.gpsimd.tensor_scalar(
                    p2[:, :], xt[:, 2 : N + 2], wk_sb[:, c, 2:3], None, MUL,
                )
                nc.vector.scalar_tensor_tensor(
                    out=midk[:, :], in0=xt[:, 1 : N + 1], scalar=wk_sb[:, c, 1:2],
                    in1=midk[:, :], op0=MUL, op1=ADD,
                )
                nc.vector.tensor_add(r32(kt[:, :]), midk[:, :], p2[:, :])
            qT.append(qt)
            kT.append(kt)

        def vtrans_block(vT, ni, use_act):
            va = va_pool.tile([128, HEAD, D + 1], F32)
            for hb in range(2):
                tp = ps_sc.tile([128, FEAT // 2], F32, tag="sc")
                for ci in range(NCH // 2):
                    cc = hb * 3 + ci
                    nc.tensor.transpose(
                        out=r32(tp[:, ci * 128 : (ci + 1) * 128]),
                        in_=r32(vT[cc][:, ni * 128 : (ni + 1) * 128]),
                        identity=r32(ident_f[:, :]),
                    )
                dstv = r32(va[:, hb * 6 : hb * 6 + 6, 0:D])
                srcv = tp[:, :].rearrange("p (h d) -> p h d", h=HEAD // 2)
                if use_act:
                    nc.vector.tensor_copy(out=dstv, in_=srcv)
                else:
                    nc.scalar.copy(out=dstv, in_=srcv)
            nc.vector.tensor_copy(out=r32(va[:, :, D : D + 1]), in_=ones_c[...])
            return va

        def attn_stageA(qT, kT, v_aug, pair, state, filler=None,
                        post_scores=None):
            def fill(k=1):
                if filler is not None:
                    for _ in range(k):
                        step = next(filler, None)
                        if step is None:
                            return
                        step()

            banks = {}
            exps = {0: [], 1: []}
            for half in (0, 1):
                hp = slice(64 * half, 64 * half + 64)
                for jb in range(NJB):
                    sc = ps_sc.tile([128, N], F32, tag="sc")
                    nc.tensor.matmul(
                        out=sc[:, :],
                        lhsT=r32(kT[pair][hp, jb * 128 : (jb + 1) * 128]),
                        rhs=r32(qT[pair][hp, :]),
                        start=True,
                        stop=True,
                    )
                    ex = exp_pool.tile([128, N], F32)
                    nc.scalar.activation(
                        out=r32(ex[:, :]), in_=sc[:, :],
                        func=mybir.ActivationFunctionType.Exp,
                    )
                    exps[half].append(ex)
                    fill(1)
            if post_scores is not None:
                for ps_fn in post_scores:
                    ps_fn()
            for half in (0, 1):
                h = 2 * pair + half
                bank = ps_attn.tile([D + 1, N], F32, tag="bank", name="bank")
                for jc in range(NJB):
                    nc.tensor.matmul(
                        out=bank[:, :],
                        lhsT=r32(v_aug[jc][:, h, :]),
                        rhs=r32(exps[half][jc][:, :]),
                        start=(jc == 0),
                        stop=(jc == NJB - 1),
                    )
                banks[half] = bank
            state[pair] = banks

        def attn_stageB(pair, state, attnT, brc_dve=False):
            banks = state.pop(pair)
            rrow = rr_pool.tile([1, 1024], F32)
            at = at_pool.tile([128, N], BF16)
            brc_sb = bs_pool.tile([D, 1024], F32)
            for half in (0, 1):
                cs = slice(512 * half, 512 * half + 512)
                nc.vector.reciprocal(
                    out=rrow[0:1, cs], in_=banks[half][D : D + 1, :]
                )
                nc.gpsimd.partition_broadcast(brc_sb[:, cs], rrow[0:1, cs])
                # odd half writes partitions 64..127 directly (partition shift)
                ps = slice(0, D) if half == 0 else slice(D, 128)
                nc.vector.tensor_mul(
                    at[ps, :], banks[half][0:D, :], brc_sb[:, cs]
                )
            attnT.append(at)

        def outproj_block(attnT, b, nb, tailmode=False):
            for step in outproj_steps(attnT, b, nb, tailmode):
                step()

        def outproj_steps(attnT, b, nb, tailmode=False):
            """Emission steps for one out-projection token block. In tailmode
            the two segments live in separate 1-bank sc-ring tiles (free in
            the tail) so consecutive blocks double-buffer; otherwise one
            2-bank ps_big tile. Bias is added at eviction via bo_bc."""
            state = {}
            ot = ot_pool.tile([128, FEAT], F32, name="ot")
            segs = ((0, 0, 512), (1, 512, FEAT))

            def alloc():
                if tailmode:
                    state[0] = ps_sc.tile([128, 512], F32, tag="sc", name="pja")
                    state[1] = ps_sc.tile([128, 512], F32, tag="sc", name="pjb")
                else:
                    pj = ps_big.tile([128, 1024], F32, tag="big", name="pj")
                    state[0] = pj[:, 0:512]
                    state[1] = pj[:, 512:1024]

            yield alloc
            for seg, lo, hi in segs:
                if tailmode:
                    def bias(seg=seg, lo=lo, hi=hi):
                        nc.tensor.matmul(
                            out=state[seg][:, 0 : hi - lo],
                            lhsT=ones_row[0:1, :],
                            rhs=bo_sb[0:1, lo:hi],
                            start=True,
                            stop=False,
                        )

                    yield bias
                for fc in range(NCH):
                    def acc(fc=fc, seg=seg, lo=lo, hi=hi):
                        tgt = state[seg]
                        nc.tensor.matmul(
                            out=tgt[:, 0 : hi - lo],
                            lhsT=attnT[fc][:, nb * 128 : (nb + 1) * 128],
                            rhs=woT_sb[fc][:, lo:hi],
                            start=(fc == 0 and not tailmode),
                            stop=(fc == NCH - 1),
                        )

                    yield acc

                def evict(seg=seg, lo=lo, hi=hi):
                    if tailmode:
                        if seg == 0:
                            nc.scalar.copy(out=ot[:, lo:hi],
                                           in_=state[seg][:, 0 : hi - lo])
                        else:
                            nc.vector.tensor_copy(
                                out=ot[:, lo:hi],
                                in_=state[seg][:, 0 : hi - lo],
                            )
                        # per-segment store: the first half ships while the
                        # second segment is still accumulating
                        dst = bass.AP(
                            tensor=out_ap.tensor,
                            offset=b * N * FEAT + nb * 128 * FEAT + lo,
                            ap=[[FEAT, 128], [1, hi - lo]],
                        )
                        nc.sync.dma_start(out=dst, in_=ot[:, lo:hi])
                    else:
                        nc.vector.scalar_tensor_tensor(
                            out=ot[:, lo:hi], in0=state[seg][:, 0 : hi - lo],
                            scalar=1.0, in1=bo_bc[:, lo:hi], op0=MUL, op1=ADD,
                        )

                yield evict

            def store():
                if tailmode:
                    return
                dst = bass.AP(
                    tensor=out_ap.tensor,
                    offset=b * N * FEAT + nb * 128 * FEAT,
                    ap=[[FEAT, 128], [1, FEAT]],
                )
                nc.sync.dma_start(out=dst, in_=ot[:, :])

            yield store

        def outproj_filler(attnT, b):
            for nb in range(NJB):
                yield from outproj_steps(attnT, b, nb)

        # ---------- emission schedule ----------
        cw_sb = consts.tile([128, NCH, 12], F32)
        bo_sb = consts.tile([1, FEAT], BF16)
        nc.sync.dma_start(out=cw_sb[...], in_=cw_d.ap())
        nc.sync.dma_start(out=bo_sb[...], in_=bo_d.ap())
        wq_sb = cw_sb[:, :, 0:3]
        wk_sb = cw_sb[:, :, 3:6]
        wv_sb = cw_sb[:, :, 6:9]
        bq_sb = cw_sb[:, :, 9:10]
        bk_sb = cw_sb[:, :, 10:11]
        bv_sb = cw_sb[:, :, 11:12]

        bo_bc = consts.tile([128, FEAT], F32)
        for seg, lo, hi in ((0, 0, 512), (1, 512, FEAT)):
            # broadcast bo via attention-bank psum slots so the big ring's
            # first slot stays free for the first x transposes
            pj0 = ps_attn.tile([128, hi - lo], F32, tag="bank", name="pj0")
            nc.tensor.matmul(
                out=pj0[:, :],
                lhsT=ones_row[0:1, :],
                rhs=bo_sb[0:1, lo:hi],
                start=True,
                stop=True,
            )
            nc.scalar.copy(out=bo_bc[:, lo:hi], in_=pj0[:, :])

        xc0 = [load_x_chunk(0, c) for c in range(NCH)]
        xtok1 = load_x(1)

        woT_sb = []
        for fc in range(NCH):
            t = consts.tile([128, FEAT], BF16, tag=f"woT{fc}")
            nc.sync.dma_start(out=t[:, :], in_=woT_d.ap()[fc * 128 : (fc + 1) * 128, :])
            woT_sb.append(t)



        q0, k0, v0, xts0 = [], [], [], []
        for c in range(NCH):
            conv_xv(xc0[c], c, v0, xts0, use_act=True)
        va0 = [vtrans_block(v0, ni, use_act=True) for ni in range(NJB)]
        conv_qk(xts0, 0, q0, k0, use_act=True)

        # attention(0) starts as soon as chunk 0's q/k and va0 are out;
        # batch-1 x/v conv and batch-0's remaining q/k convs fill the gaps
        q1, k1, v1, xts1 = [], [], [], []
        at0 = []
        st0 = {}
        for i in range(NCH):
            if i + 1 < NCH:
                conv_qk(xts0, i + 1, q0, k0, use_act=True)
            pb = [] if i == 0 else [
                (lambda j=i - 1: attn_stageB(j, st0, at0))
            ]
            attn_stageA(q0, k0, va0, i, st0, post_scores=pb)
            conv_xv(xtok1, i, v1, xts1, use_act=False, evict_act=True)
        attn_stageB(NCH - 1, st0, at0)

        # attention(1) with outproj(0) threaded into PE-queue gaps; the
        # last half of batch-1's v transposes interleaves into pair 0 so the
        # exp pipeline restarts sooner at the window boundary
        va1 = [vtrans_block(v1, ni, use_act=False) for ni in range(2)]
        conv_qk(xts1, 0, q1, k1, use_act=False)
        at1 = []
        st1 = {}
        fill0 = outproj_filler(at0, 0)

        def _rest_vtrans():
            va1.extend(
                vtrans_block(v1, ni, use_act=False) for ni in range(2, NJB)
            )

        for i in range(NCH):
            if i + 1 < NCH:
                conv_qk(xts1, i + 1, q1, k1, use_act=False)
            pb = []
            if i == 0:
                pb.append(_rest_vtrans)
            if i >= 1:
                pb.append(lambda j=i - 1: attn_stageB(j, st1, at1, brc_dve=True))
            attn_stageA(q1, k1, va1, i, st1, filler=fill0, post_scores=pb)
        attn_stageB(NCH - 1, st1, at1, brc_dve=True)
        for step in fill0:
            step()
        for nb in range(NJB):
            outproj_block(at1, 1, nb, tailmode=True)

    nc.compile()
    _PROG_CACHE["nc"] = nc
    return nc


def host_inputs(x, wq, bq, wk, bk, wv, bv, wo, bo):
    """Per-core input maps. Weight layout transforms + 1/sqrt(F) fold into q."""
    import ml_dtypes

    s = 1.0 / np.sqrt(np.float32(FEAT))

    def taps(w):  # (F,1,K) -> (128, NCH, K)
        return np.ascontiguousarray(
            w[:, 0, :].reshape(NCH, 128, KS).transpose(1, 0, 2)
        ).astype(np.float32)

    def cols(v):  # (F,) -> (128, NCH)
        return np.ascontiguousarray(v.reshape(NCH, 128).T).astype(np.float32)

    cw = np.concatenate(
        [taps(wq) * s, taps(wk), taps(wv),
         (cols(bq) * s)[:, :, None], cols(bk)[:, :, None], cols(bv)[:, :, None]],
        axis=2,
    ).astype(np.float32)
    shared = {
        "cw": np.ascontiguousarray(cw),
        "woT": np.ascontiguousarray(wo.T).astype(ml_dtypes.bfloat16),
        "bo": np.ascontiguousarray(bo.reshape(1, FEAT)).astype(ml_dtypes.bfloat16),
    }
    return [
        {"x": np.ascontiguousarray(x[c * B_LOC : (c + 1) * B_LOC]).astype(np.float32),
         **shared}
        for c in range(NCORES)
    ]


def kernel(x, wq, bq, wk, bk, wv, bv, wo, bo):
    from concourse.bass_utils import run_bass_kernel_spmd

    nc = build_program()
    x = np.asarray(x)
    in_maps = host_inputs(
        x, np.asarray(wq), np.asarray(bq), np.asarray(wk), np.asarray(bk),
        np.asarray(wv), np.asarray(bv), np.asarray(wo), np.asarray(bo),
    )
    res = run_bass_kernel_spmd(nc, in_maps, list(range(NCORES)))
    out = np.concatenate([res.results[c]["out"] for c in range(NCORES)], axis=0)
    return out.astype(np.float32)
